# revision 12
# baseline (speedup 1.0000x reference)
"""Multi-head attention kernel for Trainium2, 8 NeuronCores.

Problem: x [2, 2048, 1024], w_qkv [1024, 3072], w_proj [1024, 1024],
b_proj [1024] -> out [2, 2048, 1024]  (16 heads, head_dim 64, eval mode).

Sharding: core c in 0..7 -> batch b = c//4, head-group g = c%4 (4 heads).
Each core computes qkv projections for its 4 heads over the full sequence,
attention (scores -> softmax -> AV) for its heads, and a partial output
projection through its heads' 256 rows of w_proj. The host sums the 4
partials per batch and adds the bias (tensor-parallel unshard).

Perf notes (measured on this part):
 - matmul floor ~404 ns per N=512 op (bf16), f32r ~666 ns -> bf16 operands.
 - consecutive matmuls accumulating into the SAME psum bank run ~2x slower
   (RMW serialization) -> all accumulation chains alternate between two
   psum tiles/banks.
 - base-partition switches (lhsT at 0 vs 64) cost ~140 ns per switch and
   give no concurrency (probe: alternating 454 ns/op vs uniform 313);
   score/AV matmuls are therefore grouped into same-base runs.
 - psum->sbuf copies split across DVE and ScalarE so both accumulation
   slots release in parallel during the PE-bound qkv/proj phases.
 - softmax: no max-subtraction needed (scores ~N(0,1)); denominator
   comes free from a ones-column appended to V; the reciprocal is
   broadcast across partitions by GpSimd (f32 precision).
"""

import sys
from contextlib import ExitStack

import numpy as np

if "/opt/trn_rl_repo" not in sys.path:
    sys.path.insert(0, "/opt/trn_rl_repo")

import ml_dtypes
import concourse.bacc as bacc
import concourse.mybir as mybir
import concourse.tile as tile
from concourse.bass_utils import run_bass_kernel_spmd
from concourse.masks import make_identity

F32 = mybir.dt.float32
F32R = mybir.dt.float32r
BF16 = mybir.dt.bfloat16
AF = mybir.ActivationFunctionType

B, N, D = 2, 2048, 1024
H, HD = 16, 64
SCALE = HD ** -0.5
NCORES = 8
GROUP = 4          # cores per batch
HC = H // GROUP    # heads per core = 4
DC = HC * HD       # qkv out-dim slice per core = 256
QI_W = 1024        # attention qi tile width
NK = N // 128      # 16 kj chunks
VS_W = HC * 65     # v_store width per kj chunk (4 heads x (64 v + 1 ones))
MM_N = 512         # qkv/proj matmul free-dim chunk
SC_N = 512         # scores matmul free-dim chunk (complete groups)
AV_N = 512         # AV accumulation region = full psum bank


def _chunk_order(step=None):
    """Chunk starts covering [0, 1024) in steps, ordered so
    consecutive chunks land in different psum banks (bank = 512 f32)."""
    if step is None:
        step = MM_N
    starts = list(range(0, QI_W, step))
    lo = [s0 for s0 in starts if s0 < 512]
    hi = [s0 for s0 in starts if s0 >= 512]
    order = []
    for a, b in zip(lo, hi):
        order.extend([a, b])
    return order


def _build_program(iters=1, num_devices=NCORES):
    nc = bacc.Bacc("TRN2", target_bir_lowering=False, debug=False,
                   num_devices=num_devices)
    xT = nc.dram_tensor("xT", [D, N], BF16, kind="ExternalInput").ap()
    wqkv = nc.dram_tensor("wqkv", [D, 3 * DC], BF16, kind="ExternalInput").ap()
    wproj = nc.dram_tensor("wproj", [DC, D], BF16, kind="ExternalInput").ap()
    y = nc.dram_tensor("y", [N, D], F32, kind="ExternalOutput").ap()

    with tile.TileContext(nc) as tc, ExitStack() as ctx:
        pools = _make_pools(tc, ctx)
        for _ in range(iters):
            _emit(nc, tc, pools, xT, wqkv, wproj, y)
    nc.compile()
    return nc


def _make_pools(tc, ctx):
    p = {}
    p["const"] = ctx.enter_context(tc.tile_pool(name="const", bufs=1))
    p["xt"] = ctx.enter_context(tc.tile_pool(name="xt", bufs=8))
    p["wq"] = ctx.enter_context(tc.tile_pool(name="wq", bufs=8))
    p["qk"] = ctx.enter_context(tc.tile_pool(name="qk", bufs=4))
    p["vt"] = ctx.enter_context(tc.tile_pool(name="vt", bufs=1))
    p["vs"] = ctx.enter_context(tc.tile_pool(name="vs", bufs=1))
    p["expp"] = ctx.enter_context(tc.tile_pool(name="expp", bufs=4))
    p["outp"] = ctx.enter_context(tc.tile_pool(name="outp", bufs=2))
    p["nrm"] = ctx.enter_context(tc.tile_pool(name="nrm", bufs=2))
    p["wpj"] = ctx.enter_context(tc.tile_pool(name="wpj", bufs=2))
    p["ysb"] = ctx.enter_context(tc.tile_pool(name="ysb", bufs=2))
    # PSUM: sc 2x[128,1024] = 4 banks; avmm 2x 2-bank slots = 4 banks
    p["scps"] = ctx.enter_context(tc.tile_pool(name="scps", bufs=2, space="PSUM"))
    p["avmm"] = ctx.enter_context(tc.tile_pool(name="avmm", bufs=2, space="PSUM"))
    return p


def _emit(nc, tc, pools, xT, wqkv, wproj, y):
    mult = mybir.AluOpType.mult
    const = pools["const"]
    qk_p = pools["qk"]
    exp_p = pools["expp"]
    nrm_p = pools["nrm"]
    sc_ps = pools["scps"]
    av_ps = pools["avmm"]
    mm_ps = pools["avmm"]

    # ---------------- constants ----------------
    ident = const.tile([128, 128], F32)
    make_identity(nc, ident[:])
    ones_b = const.tile([128, 64], BF16)
    nc.vector.memset(ones_b[:], 1.0)

    # ---------------- load x and weights ----------------
    xt_sb = []
    for d in range(8):
        t = pools["xt"].tile([128, N], BF16, tag="xt")
        nc.sync.dma_start(t[:], xT[d * 128:(d + 1) * 128, :])
        xt_sb.append(t)
    wq_sb = []
    for d in range(8):
        t = pools["wq"].tile([128, 3 * DC], BF16, tag="wq")
        nc.sync.dma_start(t[:], wqkv[d * 128:(d + 1) * 128, :])
        wq_sb.append(t)
    wpj_sb = []
    for k in range(2):
        t = pools["wpj"].tile([128, D], BF16, tag="wpj")
        nc.sync.dma_start(t[:], wproj[k * 128:(k + 1) * 128, :])
        wpj_sb.append(t)

    # v_store: per kj-chunk, per head: 64 v columns + a ones column
    v_store = pools["vs"].tile([128, NK * VS_W], BF16)
    vview = v_store[:].rearrange("p (c h x) -> p c h x", c=NK, h=HC)
    nc.vector.tensor_copy(
        vview[:, :, :, 64:65],
        ones_b[:, 0:NK * HC].rearrange("p (c h x) -> p c h x", c=NK, x=1),
    )

    outT = []
    for _i in range(2):
        outT_t = pools["outp"].tile([128, N], BF16, tag="outT")
        outT.append(outT_t)

    def qkv_pair(p):
        """qkv matmuls for head-pair p. nq-pairs interleave two psum
        accumulation chains so consecutive matmuls hit different banks."""
        qT = qk_p.tile([128, N], BF16, tag="qk")
        kT = qk_p.tile([128, N], BF16, tag="qk")
        vT = pools["vt"].tile([128, N], F32, tag="vt")
        for kind, dst in ((0, qT), (1, kT), (2, vT)):
            off = kind * DC + p * 128
            for nq2 in range(2):
                ps0 = mm_ps.tile([128, 512], F32, tag="avmm")
                ps1 = mm_ps.tile([128, 512], F32, tag="avmm")
                for cj in range(0, 512, MM_N):
                    for d in range(8):
                        for j, ps in ((0, ps0), (1, ps1)):
                            nq = nq2 * 2 + j
                            nc.tensor.matmul(
                                ps[:, cj:cj + MM_N],
                                wq_sb[d][:, off:off + 128],
                                xt_sb[d][:, nq * 512 + cj:nq * 512 + cj + MM_N],
                                start=(d == 0), stop=(d == 7))
                nq = nq2 * 2
                nc.vector.tensor_copy(dst[:, nq * 512:(nq + 1) * 512], ps0[:])
                nc.scalar.copy(dst[:, (nq + 1) * 512:(nq + 2) * 512], ps1[:])
        # transpose vT pair-block into v_store (v rows onto partitions)
        for cj in range(NK):
            tp = mm_ps.tile([128, 128], F32, tag="avmm")
            nc.tensor.transpose(tp[:], vT[:, cj * 128:(cj + 1) * 128], ident[:])
            dst = v_store[:, cj * VS_W + p * 130: cj * VS_W + p * 130 + 130]
            nc.vector.tensor_copy(
                dst.rearrange("p (h x) -> p h x", x=65)[:, :, 0:64],
                tp[:].rearrange("p (h x) -> p h x", x=64))
        return qT, kT

    def attention_pair(p, qT, kT):
        """Both heads of pair p together: score matmuls for head A (lhsT at
        partitions 0:64) and head B (64:128) are issued adjacently ->
        distinct PE row groups; AV chains alternate between the two av
        psum tiles (different banks)."""
        for half in range(2):
            q0 = half * QI_W
            avA = av_ps.tile([65, QI_W], F32, tag="avmm")
            avB = av_ps.tile([65, QI_W], F32, tag="avmm")
            for kj in range(NK):
                scA = sc_ps.tile([128, QI_W], F32, tag="sc")
                scB = sc_ps.tile([128, QI_W], F32, tag="sc")
                for i in _chunk_order(SC_N):
                    nc.tensor.matmul(
                        scA[:, i: i + SC_N],
                        kT[0:64, kj * 128:(kj + 1) * 128],
                        qT[0:64, q0 + i: q0 + i + SC_N],
                        start=True, stop=True)
                for i in _chunk_order(SC_N):
                    nc.tensor.matmul(
                        scB[:, i: i + SC_N],
                        kT[64:128, kj * 128:(kj + 1) * 128],
                        qT[64:128, q0 + i: q0 + i + SC_N],
                        start=True, stop=True)
                exA = exp_p.tile([128, QI_W], BF16, tag="exp")
                exB = exp_p.tile([128, QI_W], BF16, tag="exp")
                nc.scalar.activation(exA[:], scA[:], AF.Exp)
                nc.scalar.activation(exB[:], scB[:], AF.Exp)
                vcA = kj * VS_W + (2 * p % HC) * 65
                vcB = kj * VS_W + ((2 * p + 1) % HC) * 65
                for i in _chunk_order(AV_N):
                    nc.tensor.matmul(
                        avA[:, i: i + AV_N],
                        v_store[:, vcA:vcA + 65],
                        exA[:, i: i + AV_N],
                        start=(kj == 0), stop=(kj == NK - 1))
                for i in _chunk_order(AV_N):
                    nc.tensor.matmul(
                        avB[:, i: i + AV_N],
                        v_store[:, vcB:vcB + 65],
                        exB[:, i: i + AV_N],
                        start=(kj == 0), stop=(kj == NK - 1))
            for hh, av in ((0, avA), (1, avB)):
                # normalize: out = av[0:64] * bcast(1 / av[64])
                rs = nrm_p.tile([1, QI_W], F32, tag="rs")
                nc.vector.tensor_copy(rs[:], av[64:65, :])
                rc = nrm_p.tile([1, QI_W], F32, tag="rc")
                nc.vector.reciprocal(rc[:], rs[:])
                bc = nrm_p.tile([64, QI_W], F32, tag="bc")
                nc.gpsimd.partition_broadcast(bc[:], rc[:])
                tmp = nrm_p.tile([64, QI_W], BF16, tag="tmp")
                nc.vector.tensor_tensor(tmp[:], av[0:64, :], bc[:], mult)
                nc.vector.tensor_copy(
                    outT[p][hh * 64:(hh + 1) * 64, q0:q0 + QI_W], tmp[:])

    for p in range(2):
        qT, kT = qkv_pair(p)
        attention_pair(p, qT, kT)

    # ---------------- partial output projection ----------------
    for m in range(N // 128):
        ysb = pools["ysb"].tile([128, D], F32, tag="ysb")
        ps0 = mm_ps.tile([128, 512], F32, tag="avmm")
        ps1 = mm_ps.tile([128, 512], F32, tag="avmm")
        for cj in range(0, 512, MM_N):
            for kd in range(2):
                for o, ps in ((0, ps0), (1, ps1)):
                    nc.tensor.matmul(
                        ps[:, cj:cj + MM_N],
                        outT[kd][:, m * 128:(m + 1) * 128],
                        wpj_sb[kd][:, o * 512 + cj:o * 512 + cj + MM_N],
                        start=(kd == 0), stop=(kd == 1))
        nc.vector.tensor_copy(ysb[:, 0:512], ps0[:])
        nc.scalar.copy(ysb[:, 512:1024], ps1[:])
        nc.sync.dma_start(y[m * 128:(m + 1) * 128, :], ysb[:])


_NC_CACHE = None


def _get_program():
    global _NC_CACHE
    if _NC_CACHE is None:
        _NC_CACHE = _build_program()
    return _NC_CACHE


def shard_inputs(x, w_qkv, w_proj, b_proj):
    """Build the 8 per-core input maps."""
    x = np.asarray(x, dtype=np.float32)
    w_qkv = np.asarray(w_qkv, dtype=np.float32)
    w_proj = np.asarray(w_proj, dtype=np.float32)
    bf = ml_dtypes.bfloat16
    in_maps = []
    xTs = [np.ascontiguousarray(x[b].T).astype(bf) for b in range(B)]
    for c in range(NCORES):
        b, g = divmod(c, GROUP)
        wq = w_qkv[:, g * DC:(g + 1) * DC] * np.float32(SCALE)
        wk = w_qkv[:, D + g * DC: D + (g + 1) * DC]
        wv = w_qkv[:, 2 * D + g * DC: 2 * D + (g + 1) * DC]
        in_maps.append({
            "xT": xTs[b],
            "wqkv": np.ascontiguousarray(
                np.concatenate([wq, wk, wv], axis=1)).astype(bf),
            "wproj": np.ascontiguousarray(
                w_proj[g * DC:(g + 1) * DC, :]).astype(bf),
        })
    return in_maps


def kernel(x, w_qkv, w_proj, b_proj):
    nc = _get_program()
    in_maps = shard_inputs(x, w_qkv, w_proj, b_proj)
    br = run_bass_kernel_spmd(nc, in_maps, core_ids=list(range(NCORES)))
    b_proj = np.asarray(b_proj, dtype=np.float32)
    out = np.empty((B, N, D), dtype=np.float32)
    for b in range(B):
        acc = br.results[4 * b]["y"].copy()
        for g in range(1, GROUP):
            acc += br.results[4 * b + g]["y"]
        out[b] = acc + b_proj
    return out


if __name__ == "__main__":
    rng = np.random.default_rng(0)
    x = rng.standard_normal((B, N, D), dtype=np.float32)
    w_qkv = rng.standard_normal((D, 3 * D), dtype=np.float32) * D ** -0.5
    w_proj = rng.standard_normal((D, D), dtype=np.float32) * D ** -0.5
    b_proj = rng.standard_normal((D,), dtype=np.float32) * 0.01
    got = kernel(x=x, w_qkv=w_qkv, w_proj=w_proj, b_proj=b_proj)
    qkv = (x.reshape(B * N, D) @ w_qkv).reshape(B, N, 3, H, HD)
    qkv = np.transpose(qkv, (2, 0, 3, 1, 4))
    q, k, v = qkv[0], qkv[1], qkv[2]
    s = np.einsum("bhqd,bhkd->bhqk", q, k) * SCALE
    s = s - s.max(-1, keepdims=True)
    e = np.exp(s)
    a = e / e.sum(-1, keepdims=True)
    o = np.einsum("bhqk,bhkd->bhqd", a, v)
    o = np.transpose(o, (0, 2, 1, 3)).reshape(B, N, D)
    want = o @ w_proj + b_proj
    err = np.abs(got - want)
    rel = err.max() / np.abs(want).max()
    print(f"absmax {err.max():.4e} rel-vs-absmax {rel:.4e} "
          f"rms-rel {np.sqrt((err**2).mean()/ (want**2).mean()):.4e}")


# revision 13
# speedup vs baseline: 1.1504x; 1.1504x over previous
"""Multi-head attention kernel for Trainium2, 8 NeuronCores.

Problem: x [2, 2048, 1024], w_qkv [1024, 3072], w_proj [1024, 1024],
b_proj [1024] -> out [2, 2048, 1024]  (16 heads, head_dim 64, eval mode).

Sharding: core c in 0..7 -> batch b = c//4, head-group g = c%4 (4 heads).
Each core computes qkv projections for its 4 heads over the full sequence,
attention (scores -> softmax -> AV) for its heads, and a partial output
projection through its heads' 256 rows of w_proj. The host sums the 4
partials per batch and adds the bias (tensor-parallel unshard).

Perf notes (measured on this part):
 - matmul floor ~404 ns per N=512 op (bf16), f32r ~666 ns -> bf16 operands.
 - consecutive matmuls accumulating into the SAME psum bank run ~2x slower
   (RMW serialization) -> all accumulation chains alternate between two
   psum tiles/banks.
 - base-partition switches (lhsT at 0 vs 64) cost ~140 ns per switch and
   give no concurrency (probe: alternating 454 ns/op vs uniform 313);
   score/AV matmuls are therefore grouped into same-base runs.
 - psum->sbuf copies split across DVE and ScalarE so both accumulation
   slots release in parallel during the PE-bound qkv/proj phases.
 - softmax: no max-subtraction needed (scores ~N(0,1)); denominator
   comes free from a ones-column appended to V; the reciprocal is
   broadcast across partitions by GpSimd (f32 precision).
"""

import sys
from contextlib import ExitStack

import numpy as np

if "/opt/trn_rl_repo" not in sys.path:
    sys.path.insert(0, "/opt/trn_rl_repo")

import ml_dtypes
import concourse.bacc as bacc
import concourse.mybir as mybir
import concourse.tile as tile
from concourse.bass_utils import run_bass_kernel_spmd
from concourse.masks import make_identity

F32 = mybir.dt.float32
F32R = mybir.dt.float32r
BF16 = mybir.dt.bfloat16
AF = mybir.ActivationFunctionType

B, N, D = 2, 2048, 1024
H, HD = 16, 64
SCALE = HD ** -0.5
NCORES = 8
GROUP = 4          # cores per batch
HC = H // GROUP    # heads per core = 4
DC = HC * HD       # qkv out-dim slice per core = 256
QI_W = 1024        # attention qi tile width
NK = N // 128      # 16 kj chunks
VS_W = HC * 65     # v_store width per kj chunk (4 heads x (64 v + 1 ones))
MM_N = 512         # qkv/proj matmul free-dim chunk
SC_N = 512         # scores matmul free-dim chunk (complete groups)
AV_N = 512         # AV accumulation region = full psum bank


def _chunk_order(step=None):
    """Chunk starts covering [0, 1024) in steps, ordered so
    consecutive chunks land in different psum banks (bank = 512 f32)."""
    if step is None:
        step = MM_N
    starts = list(range(0, QI_W, step))
    lo = [s0 for s0 in starts if s0 < 512]
    hi = [s0 for s0 in starts if s0 >= 512]
    order = []
    for a, b in zip(lo, hi):
        order.extend([a, b])
    return order


def _build_program(iters=1, num_devices=NCORES):
    nc = bacc.Bacc("TRN2", target_bir_lowering=False, debug=False,
                   num_devices=num_devices)
    xT = nc.dram_tensor("xT", [D, N], BF16, kind="ExternalInput").ap()
    wqkv = nc.dram_tensor("wqkv", [D, 3 * DC], BF16, kind="ExternalInput").ap()
    wproj = nc.dram_tensor("wproj", [DC, D], BF16, kind="ExternalInput").ap()
    y = nc.dram_tensor("y", [N, D], F32, kind="ExternalOutput").ap()

    with tile.TileContext(nc) as tc, ExitStack() as ctx:
        pools = _make_pools(tc, ctx)
        for _ in range(iters):
            _emit(nc, tc, pools, xT, wqkv, wproj, y)
    nc.compile()
    return nc


def _make_pools(tc, ctx):
    p = {}
    p["const"] = ctx.enter_context(tc.tile_pool(name="const", bufs=1))
    p["xt"] = ctx.enter_context(tc.tile_pool(name="xt", bufs=8))
    p["wq"] = ctx.enter_context(tc.tile_pool(name="wq", bufs=8))
    p["qk"] = ctx.enter_context(tc.tile_pool(name="qk", bufs=4))
    p["vt"] = ctx.enter_context(tc.tile_pool(name="vt", bufs=1))
    p["vs"] = ctx.enter_context(tc.tile_pool(name="vs", bufs=1))
    p["expp"] = ctx.enter_context(tc.tile_pool(name="expp", bufs=4))
    p["outp"] = ctx.enter_context(tc.tile_pool(name="outp", bufs=2))
    p["nrm"] = ctx.enter_context(tc.tile_pool(name="nrm", bufs=2))
    p["wpj"] = ctx.enter_context(tc.tile_pool(name="wpj", bufs=2))
    p["ysb"] = ctx.enter_context(tc.tile_pool(name="ysb", bufs=2))
    # PSUM: sc 2x[128,1024] = 4 banks; avmm 2x 2-bank slots = 4 banks
    p["scps"] = ctx.enter_context(tc.tile_pool(name="scps", bufs=2, space="PSUM"))
    p["avmm"] = ctx.enter_context(tc.tile_pool(name="avmm", bufs=2, space="PSUM"))
    return p


def _emit(nc, tc, pools, xT, wqkv, wproj, y):
    mult = mybir.AluOpType.mult
    const = pools["const"]
    qk_p = pools["qk"]
    exp_p = pools["expp"]
    nrm_p = pools["nrm"]
    sc_ps = pools["scps"]
    av_ps = pools["avmm"]
    mm_ps = pools["avmm"]

    # ---------------- constants ----------------
    ident = const.tile([128, 128], F32)
    make_identity(nc, ident[:])
    ones_b = const.tile([128, 64], BF16)
    nc.vector.memset(ones_b[:], 1.0)

    # ---------------- load x and weights ----------------
    xt_sb = []
    for d in range(8):
        t = pools["xt"].tile([128, N], BF16, tag="xt")
        nc.sync.dma_start(t[:], xT[d * 128:(d + 1) * 128, :])
        xt_sb.append(t)
    wq_sb = []
    for d in range(8):
        t = pools["wq"].tile([128, 3 * DC], BF16, tag="wq")
        nc.sync.dma_start(t[:], wqkv[d * 128:(d + 1) * 128, :])
        wq_sb.append(t)
    wpj_sb = []
    for k in range(2):
        t = pools["wpj"].tile([128, D], BF16, tag="wpj")
        nc.sync.dma_start(t[:], wproj[k * 128:(k + 1) * 128, :])
        wpj_sb.append(t)

    # v_store: per kj-chunk, per head: 64 v columns + a ones column
    v_store = pools["vs"].tile([128, NK * VS_W], BF16)
    vview = v_store[:].rearrange("p (c h x) -> p c h x", c=NK, h=HC)
    nc.vector.tensor_copy(
        vview[:, :, :, 64:65],
        ones_b[:, 0:NK * HC].rearrange("p (c h x) -> p c h x", c=NK, x=1),
    )

    outT = []
    for _i in range(2):
        outT_t = pools["outp"].tile([128, N], BF16, tag="outT")
        outT.append(outT_t)

    def qkv_pair(p):
        """qkv matmuls for head-pair p. nq-pairs interleave two psum
        accumulation chains so consecutive matmuls hit different banks."""
        qT = qk_p.tile([128, N], BF16, tag="qk")
        kT = qk_p.tile([128, N], BF16, tag="qk")
        vT = pools["vt"].tile([128, N], F32, tag="vt")
        for kind, dst in ((0, qT), (1, kT), (2, vT)):
            off = kind * DC + p * 128
            ps0 = mm_ps.tile([128, 512], F32, tag="avmm")
            ps1 = mm_ps.tile([128, 512], F32, tag="avmm")
            ps2 = sc_ps.tile([128, 512], F32, tag="sc")
            ps3 = sc_ps.tile([128, 512], F32, tag="sc")
            chains = (ps0, ps1, ps2, ps3)
            for d in range(8):
                for nq, ps in enumerate(chains):
                    nc.tensor.matmul(
                        ps[:],
                        wq_sb[d][:, off:off + 128],
                        xt_sb[d][:, nq * 512:(nq + 1) * 512],
                        start=(d == 0), stop=(d == 7))
            for nq, ps in enumerate(chains):
                if nq % 2 == 0:
                    nc.vector.tensor_copy(dst[:, nq * 512:(nq + 1) * 512], ps[:])
                else:
                    nc.scalar.copy(dst[:, nq * 512:(nq + 1) * 512], ps[:])
        # transpose vT pair-block into v_store (v rows onto partitions)
        for cj in range(NK):
            tp = mm_ps.tile([128, 128], F32, tag="avmm")
            nc.tensor.transpose(tp[:], vT[:, cj * 128:(cj + 1) * 128], ident[:])
            dst = v_store[:, cj * VS_W + p * 130: cj * VS_W + p * 130 + 130]
            nc.vector.tensor_copy(
                dst.rearrange("p (h x) -> p h x", x=65)[:, :, 0:64],
                tp[:].rearrange("p (h x) -> p h x", x=64))
        return qT, kT

    def attention_pair(p, qT, kT):
        """Both heads of pair p together: score matmuls for head A (lhsT at
        partitions 0:64) and head B (64:128) are issued adjacently ->
        distinct PE row groups; AV chains alternate between the two av
        psum tiles (different banks)."""
        for half in range(2):
            q0 = half * QI_W
            avA = av_ps.tile([65, QI_W], F32, tag="avmm")
            avB = av_ps.tile([65, QI_W], F32, tag="avmm")
            for kj in range(NK):
                scA = sc_ps.tile([128, QI_W], F32, tag="sc")
                scB = sc_ps.tile([128, QI_W], F32, tag="sc")
                for i in _chunk_order(SC_N):
                    nc.tensor.matmul(
                        scA[:, i: i + SC_N],
                        kT[0:64, kj * 128:(kj + 1) * 128],
                        qT[0:64, q0 + i: q0 + i + SC_N],
                        start=True, stop=True)
                for i in _chunk_order(SC_N):
                    nc.tensor.matmul(
                        scB[:, i: i + SC_N],
                        kT[64:128, kj * 128:(kj + 1) * 128],
                        qT[64:128, q0 + i: q0 + i + SC_N],
                        start=True, stop=True)
                exA = exp_p.tile([128, QI_W], BF16, tag="exp")
                exB = exp_p.tile([128, QI_W], BF16, tag="exp")
                nc.scalar.activation(exA[:], scA[:], AF.Exp)
                nc.scalar.activation(exB[:], scB[:], AF.Exp)
                vcA = kj * VS_W + (2 * p % HC) * 65
                vcB = kj * VS_W + ((2 * p + 1) % HC) * 65
                for i in _chunk_order(AV_N):
                    nc.tensor.matmul(
                        avA[:, i: i + AV_N],
                        v_store[:, vcA:vcA + 65],
                        exA[:, i: i + AV_N],
                        start=(kj == 0), stop=(kj == NK - 1))
                for i in _chunk_order(AV_N):
                    nc.tensor.matmul(
                        avB[:, i: i + AV_N],
                        v_store[:, vcB:vcB + 65],
                        exB[:, i: i + AV_N],
                        start=(kj == 0), stop=(kj == NK - 1))
            for hh, av in ((0, avA), (1, avB)):
                # normalize: out = av[0:64] * bcast(1 / av[64])
                rs = nrm_p.tile([1, QI_W], F32, tag="rs")
                nc.vector.tensor_copy(rs[:], av[64:65, :])
                rc = nrm_p.tile([1, QI_W], F32, tag="rc")
                nc.vector.reciprocal(rc[:], rs[:])
                bc = nrm_p.tile([64, QI_W], F32, tag="bc")
                nc.gpsimd.partition_broadcast(bc[:], rc[:])
                tmp = nrm_p.tile([64, QI_W], BF16, tag="tmp")
                nc.vector.tensor_tensor(tmp[:], av[0:64, :], bc[:], mult)
                nc.vector.tensor_copy(
                    outT[p][hh * 64:(hh + 1) * 64, q0:q0 + QI_W], tmp[:])

    for p in range(2):
        qT, kT = qkv_pair(p)
        attention_pair(p, qT, kT)

    # ---------------- partial output projection ----------------
    for m2 in range(N // 256):
        ysbA = pools["ysb"].tile([128, D], F32, tag="ysb")
        ysbB = pools["ysb"].tile([128, D], F32, tag="ysb")
        ps0 = mm_ps.tile([128, 512], F32, tag="avmm")
        ps1 = mm_ps.tile([128, 512], F32, tag="avmm")
        ps2 = sc_ps.tile([128, 512], F32, tag="sc")
        ps3 = sc_ps.tile([128, 512], F32, tag="sc")
        mo = ((2 * m2, 0, ps0), (2 * m2, 1, ps1),
              (2 * m2 + 1, 0, ps2), (2 * m2 + 1, 1, ps3))
        for kd in range(2):
            for m, o, ps in mo:
                nc.tensor.matmul(
                    ps[:], outT[kd][:, m * 128:(m + 1) * 128],
                    wpj_sb[kd][:, o * 512:(o + 1) * 512],
                    start=(kd == 0), stop=(kd == 1))
        nc.vector.tensor_copy(ysbA[:, 0:512], ps0[:])
        nc.scalar.copy(ysbA[:, 512:1024], ps1[:])
        nc.vector.tensor_copy(ysbB[:, 0:512], ps2[:])
        nc.scalar.copy(ysbB[:, 512:1024], ps3[:])
        nc.sync.dma_start(y[2 * m2 * 128:(2 * m2 + 1) * 128, :], ysbA[:])
        nc.sync.dma_start(y[(2 * m2 + 1) * 128:(2 * m2 + 2) * 128, :], ysbB[:])


_NC_CACHE = None


def _get_program():
    global _NC_CACHE
    if _NC_CACHE is None:
        _NC_CACHE = _build_program()
    return _NC_CACHE


def shard_inputs(x, w_qkv, w_proj, b_proj):
    """Build the 8 per-core input maps."""
    x = np.asarray(x, dtype=np.float32)
    w_qkv = np.asarray(w_qkv, dtype=np.float32)
    w_proj = np.asarray(w_proj, dtype=np.float32)
    bf = ml_dtypes.bfloat16
    in_maps = []
    xTs = [np.ascontiguousarray(x[b].T).astype(bf) for b in range(B)]
    for c in range(NCORES):
        b, g = divmod(c, GROUP)
        wq = w_qkv[:, g * DC:(g + 1) * DC] * np.float32(SCALE)
        wk = w_qkv[:, D + g * DC: D + (g + 1) * DC]
        wv = w_qkv[:, 2 * D + g * DC: 2 * D + (g + 1) * DC]
        in_maps.append({
            "xT": xTs[b],
            "wqkv": np.ascontiguousarray(
                np.concatenate([wq, wk, wv], axis=1)).astype(bf),
            "wproj": np.ascontiguousarray(
                w_proj[g * DC:(g + 1) * DC, :]).astype(bf),
        })
    return in_maps


def kernel(x, w_qkv, w_proj, b_proj):
    nc = _get_program()
    in_maps = shard_inputs(x, w_qkv, w_proj, b_proj)
    br = run_bass_kernel_spmd(nc, in_maps, core_ids=list(range(NCORES)))
    b_proj = np.asarray(b_proj, dtype=np.float32)
    out = np.empty((B, N, D), dtype=np.float32)
    for b in range(B):
        acc = br.results[4 * b]["y"].copy()
        for g in range(1, GROUP):
            acc += br.results[4 * b + g]["y"]
        out[b] = acc + b_proj
    return out


if __name__ == "__main__":
    rng = np.random.default_rng(0)
    x = rng.standard_normal((B, N, D), dtype=np.float32)
    w_qkv = rng.standard_normal((D, 3 * D), dtype=np.float32) * D ** -0.5
    w_proj = rng.standard_normal((D, D), dtype=np.float32) * D ** -0.5
    b_proj = rng.standard_normal((D,), dtype=np.float32) * 0.01
    got = kernel(x=x, w_qkv=w_qkv, w_proj=w_proj, b_proj=b_proj)
    qkv = (x.reshape(B * N, D) @ w_qkv).reshape(B, N, 3, H, HD)
    qkv = np.transpose(qkv, (2, 0, 3, 1, 4))
    q, k, v = qkv[0], qkv[1], qkv[2]
    s = np.einsum("bhqd,bhkd->bhqk", q, k) * SCALE
    s = s - s.max(-1, keepdims=True)
    e = np.exp(s)
    a = e / e.sum(-1, keepdims=True)
    o = np.einsum("bhqk,bhkd->bhqd", a, v)
    o = np.transpose(o, (0, 2, 1, 3)).reshape(B, N, D)
    want = o @ w_proj + b_proj
    err = np.abs(got - want)
    rel = err.max() / np.abs(want).max()
    print(f"absmax {err.max():.4e} rel-vs-absmax {rel:.4e} "
          f"rms-rel {np.sqrt((err**2).mean()/ (want**2).mean()):.4e}")
